# revision 69
# baseline (speedup 1.0000x reference)
"""MoE SwiGLU FFN (8 experts, top-2) + residual + LayerNorm on 8 Trainium2 cores.

Token-parallel with host-side routing/dispatch (no collectives). The host
routes tokens, assigns each token to one of 8 cores (both of its experts'
work stays on that core), and builds per-core gathered token matrices. Device
compute runs the expert matmuls in fp8-e4m3 with DoubleRow perf mode (2 fp8
weights per PE cell, K=256 per instruction), accumulating in fp32 PSUM:

  - Wv/Wg are pre-scaled by 16/8 on host so fp8 stays in its normal range;
    the scales are folded into the SiLU input-scale and the fused
    (psg + 8*bg) * silu_v vector op that produces h in fp8 (scale 8).
  - Wo carries a 12th contraction tile equal to bo, against an all-ones h
    row, so the output bias costs no separate pass; the per-token top-2
    combine weight (and the 1/128 fp8 descale) folds into the single
    psum->SBUF scale that writes y.
  - The top-2 combine gathers each token's two expert rows by indirect DMA,
    the second gather accumulating (compute_op=add); residual + LayerNorm
    run per 128-token block, gated per block on just the experts that block
    needs so the combine overlaps the tail of the expert loop.

Per-expert capacities (max per-core subset size, 4-aligned) are baked into
the program, so lightly-loaded experts don't pay the hot expert's padding.
"""

import math
import sys

import numpy as np

for p in ("/opt/trn_rl_repo",):
    if p not in sys.path:
        sys.path.insert(0, p)

import ml_dtypes

import concourse.bass as bass
import concourse.tile as tile
from concourse import bacc, mybir
from concourse.bass_utils import run_bass_kernel_spmd

EMBED = 512
HIDDEN_RAW = 1365  # floor(2*2048/3)
HIDDEN_PAD = 1408  # padded to 11*128 (zero-padded weights)
NTILE = 11  # H k-tiles
NUM_EXPERTS = 8
NCORE = 8
TOP_K = 2
LN_EPS = 1e-5
TOK_PER_CORE = 1024
TOK_BLOCKS = TOK_PER_CORE // 128

S_WV = 16.0  # Wv host pre-scale (fp8 range)
S_WG = 8.0  # Wg host pre-scale; h comes out as 8*h
S_WO = 16.0  # Wo host pre-scale; pso = 8*16*out = 128*out
DMA_COMBINE = True  # y2 gather accumulates onto y1 via compute_op=add

F32 = mybir.dt.float32
BF16 = mybir.dt.bfloat16
FP8 = mybir.dt.float8e4
I32 = mybir.dt.int32
DR = mybir.MatmulPerfMode.DoubleRow

_NC_CACHE: dict = {}


from contextlib import contextmanager


@contextmanager
def _null_ctx():
    yield


def _route(flat: np.ndarray, router_w: np.ndarray):
    """Top-2 routing identical to jax.lax.top_k tie-breaking (stable)."""
    logits = flat.astype(np.float32) @ router_w.astype(np.float32)
    order = np.argsort(-logits, axis=-1, kind="stable")
    e1 = order[:, 0].astype(np.int64)
    e2 = order[:, 1].astype(np.int64)
    v1 = np.take_along_axis(logits, order[:, :1], -1)[:, 0]
    v2 = np.take_along_axis(logits, order[:, :2], -1)[:, 1]
    m = np.maximum(v1, v2)
    a1 = np.exp(v1 - m)
    a2 = np.exp(v2 - m)
    s = a1 + a2
    return e1, e2, (a1 / s).astype(np.float32), (a2 / s).astype(np.float32)


def _assign(e1, e2):
    """Balanced token->core assignment with per-expert targets.

    Minimizes, per expert, the max per-core subset size (which is baked as
    that expert's capacity), keeping exactly TOK_PER_CORE tokens per core.
    """
    n = e1.shape[0]
    cap = n // NCORE
    tot = np.bincount(np.concatenate([e1, e2]), minlength=NUM_EXPERTS)
    targ = np.maximum((tot + NCORE - 1) // NCORE, 1).astype(np.float64)
    pair_key = e1 * NUM_EXPERTS + e2
    bucket_order = np.argsort(pair_key, kind="stable")
    core_total = np.zeros(NCORE, np.int64)
    core_expert = np.zeros((NCORE, NUM_EXPERTS), np.int64)
    assign = np.full(n, -1, np.int64)
    c = 0
    for t in bucket_order:
        a, b = e1[t], e2[t]
        best, bestcost = -1, None
        for step in range(NCORE):
            cc = (c + step) % NCORE
            if core_total[cc] >= cap:
                continue
            cost = (
                max(
                    (core_expert[cc, a] + 1) / targ[a],
                    (core_expert[cc, b] + 1) / targ[b],
                ),
                core_total[cc],
            )
            if bestcost is None or cost < bestcost:
                best, bestcost = cc, cost
        assign[t] = best
        core_total[best] += 1
        core_expert[best, a] += 1
        core_expert[best, b] += 1
        c = (best + 1) % NCORE
    return assign, core_expert


def _fp8(a: np.ndarray) -> np.ndarray:
    return np.clip(a, -240.0, 240.0).astype(ml_dtypes.float8_e4m3fn)


def _build_nc(caps: tuple, gb: tuple, gba: tuple, growsa: tuple, growsb: tuple, ln_affine: bool) -> bass.Bass:
    """Build the SPMD Bass program for per-expert capacities `caps`.

    gb[b] = number of experts (prefix length) block b's combine depends on;
    gba[b] = same for just the first (min-expert) gather slot. growsa/growsb
    are the corresponding absolute ydram row bounds (sub-expert precision,
    since slots within each expert are ordered by owning block).
    """
    key = (caps, gb, gba, growsa, growsb, ln_affine)
    if key in _NC_CACHE:
        return _NC_CACHE[key]
    blks = tuple((c + 127) // 128 for c in caps)
    off = np.concatenate([[0], np.cumsum(caps)]).astype(int)  # xt col offsets
    off2 = np.concatenate([[0], np.cumsum([b * 128 for b in blks])]).astype(int)
    ctot = int(off[-1])
    nblk = int(sum(blks))
    nrows = 128 * nblk

    nc = bacc.Bacc(None, target_bir_lowering=False)
    xt = nc.declare_dram_parameter("xt", [128, 4, ctot], FP8, isOutput=False)
    wv = nc.declare_dram_parameter("wv", [NUM_EXPERTS, 128, 4, HIDDEN_PAD], FP8, isOutput=False)
    wg = nc.declare_dram_parameter("wg", [NUM_EXPERTS, 128, 4, HIDDEN_PAD], FP8, isOutput=False)
    wo = nc.declare_dram_parameter("wo", [NUM_EXPERTS, 128, NTILE + 1, EMBED], FP8, isOutput=False)
    bias = nc.declare_dram_parameter("bias", [NUM_EXPERTS, 128, 2 * NTILE], F32, isOutput=False)
    gam = nc.declare_dram_parameter("gamma", [128, EMBED], F32, isOutput=False)
    bet = nc.declare_dram_parameter("beta", [128, EMBED], F32, isOutput=False)
    xres = nc.declare_dram_parameter("xres", [128, TOK_BLOCKS, EMBED], F32, isOutput=False)
    idx = nc.declare_dram_parameter("idx", [128, TOK_BLOCKS, 2], I32, isOutput=False)
    wsl = nc.declare_dram_parameter("wsl", [128, nblk], F32, isOutput=False)
    out = nc.declare_dram_parameter("out", [TOK_BLOCKS, 128, EMBED], F32, isOutput=True)
    ydram = nc.dram_tensor("ydram", [nrows, EMBED], BF16)

    from contextlib import ExitStack

    with tile.TileContext(nc) as tc, ExitStack() as ctx:
        const = ctx.enter_context(tc.tile_pool(name="const", bufs=1))
        wpool = ctx.enter_context(tc.tile_pool(name="w", bufs=4))
        xpool = ctx.enter_context(tc.tile_pool(name="x", bufs=3))
        hpool = ctx.enter_context(tc.tile_pool(name="h", bufs=2))
        vgpool = ctx.enter_context(tc.tile_pool(name="vg", bufs=3))
        ypool = ctx.enter_context(tc.tile_pool(name="y", bufs=3))
        pvg = ctx.enter_context(tc.tile_pool(name="pvg", bufs=3, space="PSUM"))
        pop = ctx.enter_context(tc.tile_pool(name="pop", bufs=2, space="PSUM"))
        cpool = ctx.enter_context(tc.tile_pool(name="c", bufs=2))

        cmax = max(caps)
        gam_t = const.tile([128, EMBED], F32)
        bet_t = const.tile([128, EMBED], F32)
        eps_t = const.tile([128, 1], F32)
        nc.vector.memset(eps_t, LN_EPS)
        idx_t = const.tile([128, TOK_BLOCKS, 2], I32)
        wsl_t = const.tile([128, nblk], F32)
        xres_t = const.tile([128, TOK_BLOCKS, EMBED], F32)
        warm_t = const.tile([128, 1], F32)
        nc.scalar.activation(out=warm_t, in_=eps_t, func=mybir.ActivationFunctionType.Silu, bias=0.0, scale=1.0)
        zero_t = const.tile([128, EMBED], BF16)
        nc.vector.memset(zero_t, 0.0)

        # per-block emission points: the slot-0 (min-expert) gather of block b
        # runs after expert gba[b]-1, the slot-1 gather after gb[b]-1, and
        # the vector LN chain one expert later still -- so the ~4us indirect
        # gather transfer never stalls the vector queue's FIFO head
        # trigger each gather right after the specific ydram write that
        # satisfies its row bound, so the transfer overlaps remaining compute
        nwrites = int(sum(blks))
        gather_a_at = [[] for _ in range(nwrites + 1)]
        gather_b_at = [[] for _ in range(nwrites + 1)]
        ln_after = [[] for _ in range(NUM_EXPERTS)]
        out_after = [[] for _ in range(NUM_EXPERTS)]
        for b in range(TOK_BLOCKS):
            gather_a_at[-(-int(growsa[b]) // 128)].append(b)
            gather_b_at[-(-int(growsb[b]) // 128)].append(b)
            ln_e = min(gb[b], NUM_EXPERTS - 1)
            ln_after[ln_e].append(b)
            # out-write another expert later: by then nrm is guaranteed
            # ready, so the issue never stalls the scalar queue's head
            out_after[min(ln_e + 1, NUM_EXPERTS - 1)].append(b)
        t1pool = ctx.enter_context(tc.tile_pool(name="t1", bufs=TOK_BLOCKS))
        apool = ctx.enter_context(tc.tile_pool(name="ap", bufs=TOK_BLOCKS))
        nrmpool = ctx.enter_context(tc.tile_pool(name="nrm", bufs=TOK_BLOCKS))
        t1_tiles: dict = {}
        a_tiles: dict = {}
        nrm_tiles: dict = {}

        def gather_a(b: int):
            rows = int(growsa[b])
            t1 = t1pool.tile([128, EMBED], BF16, tag="t1")
            t1_tiles[b] = t1
            nc.gpsimd.indirect_dma_start(
                out=t1,
                out_offset=None,
                in_=ydram[:rows],
                in_offset=bass.IndirectOffsetOnAxis(ap=idx_t[:, b, 0:1], axis=0),
            )

        def gather_b(b: int):
            rows = int(growsb[b])
            t1 = t1_tiles[b]
            nc.gpsimd.indirect_dma_start(
                out=t1,
                out_offset=None,
                in_=ydram[:rows],
                in_offset=bass.IndirectOffsetOnAxis(ap=idx_t[:, b, 1:2], axis=0),
                compute_op=mybir.AluOpType.add,
            )

        def residual_add(b: int):
            # on gpsimd: waits on its own ring's gather (natural order)
            # instead of head-blocking the vector queue
            t1 = t1_tiles[b]
            a_t = apool.tile([128, EMBED], F32, tag="a")
            a_tiles[b] = a_t
            nc.gpsimd.tensor_add(out=a_t, in0=t1, in1=xres_t[:, b, :])

        def combine(b: int):
            a_t = a_tiles[b]
            stats = cpool.tile([128, 6], F32, tag="st")
            nc.vector.bn_stats(out=stats, in_=a_t)
            mv = cpool.tile([128, 2], F32, tag="mv")
            nc.vector.bn_aggr(out=mv, in_=stats)
            # rs = rsqrt(var + eps) entirely on DVE (keeps ScalarE's act
            # table pinned to Silu): bit-trick seed + 2 Newton steps
            ve = cpool.tile([128, 1], F32, tag="ve")
            nc.vector.tensor_scalar_add(out=ve, in0=mv[:, 1:2], scalar1=LN_EPS)
            hve = cpool.tile([128, 1], F32, tag="hve")
            nc.vector.tensor_scalar_mul(out=hve, in0=ve, scalar1=-0.5)
            rsi = cpool.tile([128, 1], I32, tag="rsi")
            nc.vector.tensor_scalar(
                out=rsi, in0=ve.bitcast(I32), scalar1=1, scalar2=None,
                op0=mybir.AluOpType.logical_shift_right,
            )
            nc.vector.tensor_scalar(
                out=rsi, in0=rsi, scalar1=-1, scalar2=None,
                op0=mybir.AluOpType.bitwise_xor,
            )
            nc.vector.tensor_scalar(
                out=rsi, in0=rsi, scalar1=0x5F3759DF + 1, scalar2=None,
                op0=mybir.AluOpType.add,
            )
            rs = rsi.bitcast(F32)
            t_ = cpool.tile([128, 1], F32, tag="tnw")
            for _ in range(2):
                nc.vector.tensor_mul(out=t_, in0=rs, in1=rs)
                nc.vector.tensor_scalar(
                    out=t_, in0=t_, scalar1=hve, scalar2=1.5,
                    op0=mybir.AluOpType.mult, op1=mybir.AluOpType.add,
                )
                nc.vector.tensor_mul(out=rs, in0=rs, in1=t_)
            nrm = nrmpool.tile([128, EMBED], F32, tag="nrm")
            nrm_tiles[b] = nrm
            nc.vector.tensor_scalar(
                out=nrm, in0=a_t, scalar1=mv[:, 0:1], scalar2=rs,
                op0=mybir.AluOpType.subtract, op1=mybir.AluOpType.mult,
            )
            if ln_affine:
                nc.vector.tensor_mul(out=nrm, in0=nrm, in1=gam_t)
                nc.vector.tensor_add(out=nrm, in0=nrm, in1=bet_t)

        for e in range(NUM_EXPERTS):
            C = caps[e]
            xt_t = xpool.tile([128, 4, C], FP8, tag="xt")
            b_t = wpool.tile([128, 2 * NTILE], F32, tag="bias")
            wv_t = wpool.tile([128, 4, HIDDEN_PAD], FP8, tag="wv")
            wg_t = wpool.tile([128, 4, HIDDEN_PAD], FP8, tag="wg")
            wo_t = wpool.tile([128, NTILE + 1, EMBED], FP8, tag="wo")
            o0 = int(off[e])
            if e == 0:
                # startup: spread the critical first loads across queues so
                # the first matmul pair can begin as early as possible
                nc.sync.dma_start(out=xt_t[:, :2], in_=xt[:, :2, o0 : o0 + C])
                nc.scalar.dma_start(out=wv_t[:, :1], in_=wv[e, :, :1])
                nc.gpsimd.dma_start(out=wv_t[:, 1:2], in_=wv[e, :, 1:2])
                nc.sync.dma_start(out=xt_t[:, 2:], in_=xt[:, 2:, o0 : o0 + C])
                nc.sync.dma_start(out=b_t, in_=bias[e])
                nc.scalar.dma_start(out=wv_t[:, 2:], in_=wv[e, :, 2:])
                nc.gpsimd.dma_start(out=wg_t[:, :2], in_=wg[e, :, :2])
                nc.scalar.dma_start(out=wg_t[:, 2:], in_=wg[e, :, 2:])
                nc.sync.dma_start(out=wo_t, in_=wo[e])
                nc.scalar.dma_start(out=idx_t, in_=idx[:, :, :])
                nc.scalar.dma_start(out=wsl_t, in_=wsl[:, :])
                nc.gpsimd.dma_start(out=xres_t, in_=xres[:, :, :])
                if ln_affine:
                    nc.scalar.dma_start(out=gam_t, in_=gam[:, :])
                    nc.scalar.dma_start(out=bet_t, in_=bet[:, :])
            elif e == 1:
                # scalar/gpsimd rings are still idle this early; keep the
                # sync ring short so e1's weights land before e0 compute ends
                nc.sync.dma_start(out=xt_t, in_=xt[:, :, o0 : o0 + C])
                nc.sync.dma_start(out=b_t, in_=bias[e])
                nc.scalar.dma_start(out=wv_t, in_=wv[e])
                nc.gpsimd.dma_start(out=wg_t, in_=wg[e])
                nc.sync.dma_start(out=wo_t, in_=wo[e])
            else:
                # steady state: the sync queue carries ONLY loads, so weight
                # prefetch never head-blocks behind compute-dependent DMAs
                nc.sync.dma_start(out=xt_t, in_=xt[:, :, o0 : o0 + C])
                nc.sync.dma_start(out=b_t, in_=bias[e])
                nc.sync.dma_start(out=wv_t, in_=wv[e])
                nc.sync.dma_start(out=wg_t, in_=wg[e])
                nc.sync.dma_start(out=wo_t, in_=wo[e])

            h_t = hpool.tile([128, NTILE + 1, C], FP8, tag="h")
            nc.vector.memset(h_t[:, NTILE, :], 1.0)

            for m in range(NTILE):
                psv = pvg.tile([128, cmax], F32, tag="psv")
                psg = pvg.tile([128, cmax], F32, tag="psg")
                for kk in range(2):
                    nc.tensor.matmul(
                        psv[:, :C],
                        lhsT=wv_t[:, 2 * kk : 2 * kk + 2, m * 128 : (m + 1) * 128],
                        rhs=xt_t[:, 2 * kk : 2 * kk + 2, :],
                        start=(kk == 0),
                        stop=(kk == 1),
                        perf_mode=DR,
                    )
                for kk in range(2):
                    nc.tensor.matmul(
                        psg[:, :C],
                        lhsT=wg_t[:, 2 * kk : 2 * kk + 2, m * 128 : (m + 1) * 128],
                        rhs=xt_t[:, 2 * kk : 2 * kk + 2, :],
                        start=(kk == 0),
                        stop=(kk == 1),
                        perf_mode=DR,
                    )
                v_t = vgpool.tile([128, C], BF16, tag="v")
                nc.scalar.activation(
                    out=v_t,
                    in_=psv[:, :C],
                    func=mybir.ActivationFunctionType.Silu,
                    bias=b_t[:, m : m + 1],
                    scale=1.0 / S_WV,
                )
                # h(fp8, x8) = (psg + 8*bg) * silu_v  -- one fused DVE pass
                nc.vector.scalar_tensor_tensor(
                    out=h_t[:, m, :],
                    in0=psg[:, :C],
                    scalar=b_t[:, NTILE + m : NTILE + m + 1],
                    in1=v_t,
                    op0=mybir.AluOpType.add,
                    op1=mybir.AluOpType.mult,
                )
            for blk in range(blks[e]):
                mb = min(128, C - blk * 128)
                gblk = int(off2[e]) // 128 + blk
                pso = pop.tile([128, EMBED], F32, tag="pso")
                for kk in range(6):
                    nc.tensor.matmul(
                        pso[:mb],
                        lhsT=h_t[:, 2 * kk : 2 * kk + 2, blk * 128 : blk * 128 + mb],
                        rhs=wo_t[:, 2 * kk : 2 * kk + 2, :],
                        start=(kk == 0),
                        stop=(kk == 5),
                        perf_mode=DR,
                    )
                y_t = ypool.tile([128, EMBED], BF16, tag="y")
                if e == NUM_EXPERTS - 1:
                    # tail-critical path: ScalarE is idle by now and the
                    # ydram issue follows on the same engine, skipping the
                    # vector queue's backlog entirely
                    with tc.high_priority(offset=None):
                        nc.scalar.activation(
                            out=y_t[:mb],
                            in_=pso[:mb],
                            func=mybir.ActivationFunctionType.Identity,
                            bias=0.0,
                            scale=wsl_t[:mb, gblk : gblk + 1],
                        )
                else:
                    nc.vector.tensor_scalar_mul(
                        out=y_t[:mb], in0=pso[:mb], scalar1=wsl_t[:mb, gblk : gblk + 1]
                    )
                r0 = gblk * 128
                with tc.high_priority(offset=None) if e == NUM_EXPERTS - 1 else _null_ctx():
                    nc.scalar.dma_start(out=ydram[r0 : r0 + mb, :], in_=y_t[:mb])
                if mb < 128:
                    nc.scalar.dma_start(
                        out=ydram[r0 + mb : r0 + 128, :], in_=zero_t[: 128 - mb]
                    )
                with tc.high_priority(offset=-1000000):
                    for b in gather_a_at[gblk + 1]:
                        gather_a(b)
                    for b in gather_b_at[gblk + 1]:
                        gather_b(b)
                        residual_add(b)

            with tc.high_priority(offset=-1000000):
                for b in ln_after[e]:
                    combine(b)
                for b in out_after[e]:
                    nc.scalar.dma_start(out=out[b], in_=nrm_tiles[b])

    nc.finalize()
    _NC_CACHE[key] = nc
    return nc


def prepare(x, router_w, Wv, bv, Wg, bg, Wo, bo, gamma, beta):
    """Host-side routing, balancing, and per-core input construction."""
    x = np.asarray(x)
    router_w = np.asarray(router_w, dtype=np.float32)
    Wv = np.asarray(Wv, dtype=np.float32)
    bv = np.asarray(bv, dtype=np.float32)
    Wg = np.asarray(Wg, dtype=np.float32)
    bg = np.asarray(bg, dtype=np.float32)
    Wo = np.asarray(Wo, dtype=np.float32)
    bo = np.asarray(bo, dtype=np.float32)
    gamma = np.asarray(gamma, dtype=np.float32)
    beta = np.asarray(beta, dtype=np.float32)

    orig_shape = x.shape
    flat = x.reshape(-1, EMBED).astype(np.float32)
    n = flat.shape[0]
    assert n == NCORE * TOK_PER_CORE

    e1, e2, w1, w2 = _route(flat, router_w)
    # process experts in descending popularity: the LAST-processed expert
    # gates the tail combines, so make it the least-loaded one (fewest
    # dependent tokens -> fewer tail blocks)
    cnt = np.bincount(np.concatenate([e1, e2]), minlength=NUM_EXPERTS)
    perm = np.argsort(-cnt, kind="stable")  # position -> physical expert
    inv = np.empty(NUM_EXPERTS, np.int64)
    inv[perm] = np.arange(NUM_EXPERTS)
    e1 = inv[e1]
    e2 = inv[e2]
    Wv, bv = Wv[perm], bv[perm]
    Wg, bg = Wg[perm], bg[perm]
    Wo, bo = Wo[perm], bo[perm]
    assign, core_expert = _assign(e1, e2)

    # 16-aligned: DoubleRow LDWEIGHTS requires the k-pair stride (=C) % 16 == 0
    caps = tuple(int(-16 * (-core_expert[:, e].max() // 16)) for e in range(NUM_EXPERTS))
    blks = tuple((c + 127) // 128 for c in caps)
    off = np.concatenate([[0], np.cumsum(caps)]).astype(int)
    off2 = np.concatenate([[0], np.cumsum([b * 128 for b in blks])]).astype(int)
    ctot = int(off[-1])
    nblk = int(sum(blks))

    # replicated weights, pre-tiled to [e, 128, ktiles, free], fp8 pre-scaled
    def ktile(w, s):  # [E, 512, F] -> [E, 128, 4, F] fp8 of s*w
        E_, K_, F_ = w.shape
        return np.ascontiguousarray(
            _fp8(s * w).reshape(E_, 4, 128, F_).transpose(0, 2, 1, 3)
        )

    wv_r = np.zeros((NUM_EXPERTS, EMBED, HIDDEN_PAD), np.float32)
    wv_r[:, :, :HIDDEN_RAW] = Wv
    wg_r = np.zeros((NUM_EXPERTS, EMBED, HIDDEN_PAD), np.float32)
    wg_r[:, :, :HIDDEN_RAW] = Wg
    wv_tiled = ktile(wv_r, S_WV)
    wg_tiled = ktile(wg_r, S_WG)
    # Wo: [E, H, 512] -> tiles [E, 128, 12, 512]; tile 11 = bo (bias k-tile)
    wo_r = np.zeros((NUM_EXPERTS, (NTILE + 1) * 128, EMBED), np.float32)
    wo_r[:, :HIDDEN_RAW, :] = S_WO * Wo
    wo_r[:, NTILE * 128 :, :] = bo[:, None, :]  # all 128 rows = bo
    wo_tiled = np.ascontiguousarray(
        _fp8(wo_r).reshape(NUM_EXPERTS, NTILE + 1, 128, EMBED).transpose(0, 2, 1, 3)
    )
    bv_pad = np.zeros((NUM_EXPERTS, HIDDEN_PAD), np.float32)
    bv_pad[:, :HIDDEN_RAW] = bv
    bg_pad = np.zeros((NUM_EXPERTS, HIDDEN_PAD), np.float32)
    bg_pad[:, :HIDDEN_RAW] = bg
    # bias [e, 128, 22]: cols 0..10 = bv tiles, 11..21 = 8*bg tiles
    bias_h = np.concatenate(
        [
            bv_pad.reshape(NUM_EXPERTS, NTILE, 128).transpose(0, 2, 1),
            S_WG * bg_pad.reshape(NUM_EXPERTS, NTILE, 128).transpose(0, 2, 1),
        ],
        axis=2,
    ).astype(np.float32)
    bias_h = np.ascontiguousarray(bias_h)
    gam_rep = np.ascontiguousarray(np.broadcast_to(gamma, (128, EMBED)))
    bet_rep = np.ascontiguousarray(np.broadcast_to(beta, (128, EMBED)))

    # per-core token ordering by max expert touched -> per-block gather bounds
    emin_all = np.minimum(e1, e2)
    emax_all = np.maximum(e1, e2)
    per_core_sorted = []
    gbv = np.zeros(TOK_BLOCKS, np.int64)
    gbav = np.zeros(TOK_BLOCKS, np.int64)
    for c in range(NCORE):
        tok_c = np.nonzero(assign == c)[0]
        assert tok_c.size == TOK_PER_CORE
        emax = emax_all[tok_c]
        order = np.argsort(emax, kind="stable")
        tok_c = tok_c[order]
        per_core_sorted.append(tok_c)
        emax_sorted = emax[order]
        emin_sorted = emin_all[tok_c]
        for b in range(TOK_BLOCKS):
            gbv[b] = max(gbv[b], emax_sorted[(b + 1) * 128 - 1] + 1)
            gbav[b] = max(gbav[b], emin_sorted[b * 128 : (b + 1) * 128].max() + 1)
    gb = tuple(int(v) for v in gbv)
    gba = tuple(int(v) for v in gbav)

    in_maps = []
    core_token_ids = []
    growsa_v = np.zeros(TOK_BLOCKS, np.int64)
    growsb_v = np.zeros(TOK_BLOCKS, np.int64)
    for c in range(NCORE):
        tok_c = per_core_sorted[c]
        core_token_ids.append(tok_c)
        xt_c = np.zeros((EMBED, ctot), np.float32)
        ridx = np.zeros((TOK_PER_CORE, 2), np.int64)
        wslot = np.zeros(nblk * 128, np.float32)
        ids_by_e = []
        for e in range(NUM_EXPERTS):
            sel1 = e1[tok_c] == e
            sel2 = e2[tok_c] == e
            # slots ordered by token position (= owning combine block), so
            # per-block gather bounds only cover a sub-expert row prefix
            ids = np.nonzero(sel1 | sel2)[0]
            ids_by_e.append(ids)
            cnt = ids.size
            assert cnt <= caps[e], (cnt, caps[e])
            xt_c[:, off[e] : off[e] + cnt] = flat[tok_c[ids]].T
            rows = off2[e] + np.arange(cnt)
            # slot 0 = the token's min-expert row (gatherable early),
            # slot 1 = its max-expert row
            col = (emax_all[tok_c[ids]] == e).astype(np.int64)
            ridx[ids, col] = rows
            wslot[rows] = np.where(sel1[ids], w1[tok_c[ids]], w2[tok_c[ids]])
        emax_c = emax_all[tok_c]
        emin_c = emin_all[tok_c]
        for b in range(TOK_BLOCKS):
            hi = (b + 1) * 128
            eb = int(emax_c[b * 128 : hi].max())
            ea = int(emin_c[b * 128 : hi].max())
            growsb_v[b] = max(
                growsb_v[b], off2[eb] + np.searchsorted(ids_by_e[eb], hi)
            )
            growsa_v[b] = max(
                growsa_v[b], off2[ea] + np.searchsorted(ids_by_e[ea], hi)
            )
        in_map = {
            "xt": np.ascontiguousarray(
                _fp8(xt_c).reshape(4, 128, ctot).transpose(1, 0, 2)
            ),
            "wv": wv_tiled,
            "wg": wg_tiled,
            "wo": wo_tiled,
            "bias": bias_h,
            "gamma": gam_rep,
            "beta": bet_rep,
            "xres": np.ascontiguousarray(
                flat[tok_c].reshape(TOK_BLOCKS, 128, EMBED).transpose(1, 0, 2)
            ),
            "idx": np.ascontiguousarray(
                ridx.reshape(TOK_BLOCKS, 128, 2).transpose(1, 0, 2).astype(np.int32)
            ),
            "wsl": np.ascontiguousarray(
                (wslot / 128.0).reshape(nblk, 128).T.astype(np.float32)
            ),
        }
        in_maps.append(in_map)

    growsa = tuple(int(v) for v in growsa_v)
    growsb = tuple(int(v) for v in growsb_v)
    ln_affine = not (np.all(gamma == 1.0) and np.all(beta == 0.0))
    return in_maps, caps, gb, gba, growsa, growsb, ln_affine, core_token_ids, orig_shape


def assemble(results, core_token_ids, orig_shape):
    n = NCORE * TOK_PER_CORE
    out_full = np.zeros((n, EMBED), np.float32)
    for c in range(NCORE):
        out_full[core_token_ids[c]] = results[c]["out"].reshape(TOK_PER_CORE, EMBED)
    return out_full.reshape(orig_shape)


def kernel(x, router_w, Wv, bv, Wg, bg, Wo, bo, gamma, beta):
    in_maps, caps, gb, gba, growsa, growsb, ln_affine, core_token_ids, orig_shape = prepare(
        x, router_w, Wv, bv, Wg, bg, Wo, bo, gamma, beta
    )
    nc = _build_nc(caps, gb, gba, growsa, growsb, ln_affine)
    res = run_bass_kernel_spmd(nc, in_maps, list(range(NCORE)))
    return assemble(res.results, core_token_ids, orig_shape)


# revision 73
# speedup vs baseline: 1.1042x; 1.1042x over previous
"""MoE SwiGLU FFN (8 experts, top-2) + residual + LayerNorm on 8 Trainium2 cores.

Token-parallel with host-side routing/dispatch (no collectives). The host
routes tokens, assigns each token to one of 8 cores (both of its experts'
work stays on that core), and builds per-core gathered token matrices. Device
compute runs the expert matmuls in fp8-e4m3 with DoubleRow perf mode (2 fp8
weights per PE cell, K=256 per instruction), accumulating in fp32 PSUM:

  - Wv/Wg are pre-scaled by 16/8 on host so fp8 stays in its normal range;
    the scales are folded into the SiLU input-scale and the fused
    (psg + 8*bg) * silu_v vector op that produces h in fp8 (scale 8).
  - Wo carries a 12th contraction tile equal to bo, against an all-ones h
    row, so the output bias costs no separate pass; the per-token top-2
    combine weight (and the 1/128 fp8 descale) folds into the single
    psum->SBUF scale that writes y.
  - The top-2 combine gathers each token's two expert rows by indirect DMA,
    the second gather accumulating (compute_op=add); residual + LayerNorm
    run per 128-token block, gated per block on just the experts that block
    needs so the combine overlaps the tail of the expert loop.

Per-expert capacities (max per-core subset size, 4-aligned) are baked into
the program, so lightly-loaded experts don't pay the hot expert's padding.
"""

import math
import sys

import numpy as np

for p in ("/opt/trn_rl_repo",):
    if p not in sys.path:
        sys.path.insert(0, p)

import ml_dtypes

import concourse.bass as bass
import concourse.tile as tile
from concourse import bacc, mybir
from concourse.bass_utils import run_bass_kernel_spmd

EMBED = 512
HIDDEN_RAW = 1365  # floor(2*2048/3)
HIDDEN_PAD = 1408  # padded to 11*128 (zero-padded weights)
NTILE = 11  # H k-tiles
NUM_EXPERTS = 8
NCORE = 8
TOP_K = 2
LN_EPS = 1e-5
TOK_PER_CORE = 1024
TOK_BLOCKS = TOK_PER_CORE // 128

S_WV = 16.0  # Wv host pre-scale (fp8 range)
S_WG = 8.0  # Wg host pre-scale; h comes out as 8*h
S_WO = 16.0  # Wo host pre-scale; pso = 8*16*out = 128*out
DMA_COMBINE = True  # y2 gather accumulates onto y1 via compute_op=add

F32 = mybir.dt.float32
BF16 = mybir.dt.bfloat16
FP8 = mybir.dt.float8e4
I32 = mybir.dt.int32
DR = mybir.MatmulPerfMode.DoubleRow

_NC_CACHE: dict = {}


from contextlib import contextmanager


@contextmanager
def _null_ctx():
    yield


def _route(flat: np.ndarray, router_w: np.ndarray):
    """Top-2 routing identical to jax.lax.top_k tie-breaking (stable)."""
    logits = flat.astype(np.float32) @ router_w.astype(np.float32)
    order = np.argsort(-logits, axis=-1, kind="stable")
    e1 = order[:, 0].astype(np.int64)
    e2 = order[:, 1].astype(np.int64)
    v1 = np.take_along_axis(logits, order[:, :1], -1)[:, 0]
    v2 = np.take_along_axis(logits, order[:, :2], -1)[:, 1]
    m = np.maximum(v1, v2)
    a1 = np.exp(v1 - m)
    a2 = np.exp(v2 - m)
    s = a1 + a2
    return e1, e2, (a1 / s).astype(np.float32), (a2 / s).astype(np.float32)


def _assign(e1, e2):
    """Balanced token->core assignment with per-expert targets.

    Minimizes, per expert, the max per-core subset size (which is baked as
    that expert's capacity), keeping exactly TOK_PER_CORE tokens per core.
    """
    n = e1.shape[0]
    cap = n // NCORE
    tot = np.bincount(np.concatenate([e1, e2]), minlength=NUM_EXPERTS)
    targ = np.maximum((tot + NCORE - 1) // NCORE, 1).astype(np.float64)
    pair_key = e1 * NUM_EXPERTS + e2
    bucket_order = np.argsort(pair_key, kind="stable")
    core_total = np.zeros(NCORE, np.int64)
    core_expert = np.zeros((NCORE, NUM_EXPERTS), np.int64)
    assign = np.full(n, -1, np.int64)
    c = 0
    for t in bucket_order:
        a, b = e1[t], e2[t]
        best, bestcost = -1, None
        for step in range(NCORE):
            cc = (c + step) % NCORE
            if core_total[cc] >= cap:
                continue
            cost = (
                max(
                    (core_expert[cc, a] + 1) / targ[a],
                    (core_expert[cc, b] + 1) / targ[b],
                ),
                core_total[cc],
            )
            if bestcost is None or cost < bestcost:
                best, bestcost = cc, cost
        assign[t] = best
        core_total[best] += 1
        core_expert[best, a] += 1
        core_expert[best, b] += 1
        c = (best + 1) % NCORE
    return assign, core_expert


def _fp8(a: np.ndarray) -> np.ndarray:
    return np.clip(a, -240.0, 240.0).astype(ml_dtypes.float8_e4m3fn)


def _build_nc(caps: tuple, gb: tuple, gba: tuple, growsa: tuple, growsb: tuple, ln_affine: bool) -> bass.Bass:
    """Build the SPMD Bass program for per-expert capacities `caps`.

    gb[b] = number of experts (prefix length) block b's combine depends on;
    gba[b] = same for just the first (min-expert) gather slot. growsa/growsb
    are the corresponding absolute ydram row bounds (sub-expert precision,
    since slots within each expert are ordered by owning block).
    """
    key = (caps, gb, gba, growsa, growsb, ln_affine)
    if key in _NC_CACHE:
        return _NC_CACHE[key]
    blks = tuple((c + 127) // 128 for c in caps)
    off = np.concatenate([[0], np.cumsum(caps)]).astype(int)  # xt col offsets
    off2 = np.concatenate([[0], np.cumsum([b * 128 for b in blks])]).astype(int)
    ctot = int(off[-1])
    nblk = int(sum(blks))
    nrows = 128 * nblk

    nc = bacc.Bacc(None, target_bir_lowering=False)
    xt = nc.declare_dram_parameter("xt", [128, 4, ctot], FP8, isOutput=False)
    wv = nc.declare_dram_parameter("wv", [NUM_EXPERTS, 128, 4, HIDDEN_PAD], FP8, isOutput=False)
    wg = nc.declare_dram_parameter("wg", [NUM_EXPERTS, 128, 4, HIDDEN_PAD], FP8, isOutput=False)
    wo = nc.declare_dram_parameter("wo", [NUM_EXPERTS, 128, NTILE + 1, EMBED], FP8, isOutput=False)
    bias = nc.declare_dram_parameter("bias", [NUM_EXPERTS, 128, 2 * NTILE], F32, isOutput=False)
    gam = nc.declare_dram_parameter("gamma", [128, EMBED], F32, isOutput=False)
    bet = nc.declare_dram_parameter("beta", [128, EMBED], F32, isOutput=False)
    xres = nc.declare_dram_parameter("xres", [128, TOK_BLOCKS, EMBED], F32, isOutput=False)
    idx = nc.declare_dram_parameter("idx", [128, TOK_BLOCKS, 2], I32, isOutput=False)
    wsl = nc.declare_dram_parameter("wsl", [128, nblk], F32, isOutput=False)
    out = nc.declare_dram_parameter("out", [TOK_BLOCKS, 128, EMBED], F32, isOutput=True)
    ydram = nc.dram_tensor("ydram", [nrows, EMBED], BF16)

    from contextlib import ExitStack

    with tile.TileContext(nc) as tc, ExitStack() as ctx:
        const = ctx.enter_context(tc.tile_pool(name="const", bufs=1))
        wpool = ctx.enter_context(tc.tile_pool(name="w", bufs=4))
        xpool = ctx.enter_context(tc.tile_pool(name="x", bufs=3))
        hpool = ctx.enter_context(tc.tile_pool(name="h", bufs=2))
        vgpool = ctx.enter_context(tc.tile_pool(name="vg", bufs=3))
        ypool = ctx.enter_context(tc.tile_pool(name="y", bufs=3))
        pvg = ctx.enter_context(tc.tile_pool(name="pvg", bufs=3, space="PSUM"))
        pop = ctx.enter_context(tc.tile_pool(name="pop", bufs=2, space="PSUM"))
        cpool = ctx.enter_context(tc.tile_pool(name="c", bufs=2))

        cmax = max(caps)
        gam_t = const.tile([128, EMBED], F32)
        bet_t = const.tile([128, EMBED], F32)
        eps_t = const.tile([128, 1], F32)
        nc.vector.memset(eps_t, LN_EPS)
        idx_t = const.tile([128, TOK_BLOCKS, 2], I32)
        wsl_t = const.tile([128, nblk], F32)
        xres_t = const.tile([128, TOK_BLOCKS, EMBED], F32)
        warm_t = const.tile([128, 1], F32)
        nc.scalar.activation(out=warm_t, in_=eps_t, func=mybir.ActivationFunctionType.Silu, bias=0.0, scale=1.0)
        zero_t = const.tile([128, EMBED], BF16)
        nc.vector.memset(zero_t, 0.0)

        # per-block emission points: the slot-0 (min-expert) gather of block b
        # runs after expert gba[b]-1, the slot-1 gather after gb[b]-1, and
        # the vector LN chain one expert later still -- so the ~4us indirect
        # gather transfer never stalls the vector queue's FIFO head
        # trigger each gather right after the specific ydram write that
        # satisfies its row bound, so the transfer overlaps remaining compute
        nwrites = int(sum(blks))
        gather_a_at = [[] for _ in range(nwrites + 1)]
        gather_b_at = [[] for _ in range(nwrites + 1)]
        ln_mid = [[] for _ in range(NUM_EXPERTS)]  # LN inside section's m-loop
        ln_end = [[] for _ in range(NUM_EXPERTS)]  # LN after section's phase-2
        out_after = [[] for _ in range(NUM_EXPERTS)]
        for b in range(TOK_BLOCKS):
            gather_a_at[-(-int(growsa[b]) // 128)].append(b)
            gather_b_at[-(-int(growsb[b]) // 128)].append(b)
            ln_e = min(gb[b], NUM_EXPERTS - 1)
            if gb[b] <= NUM_EXPERTS - 1:
                # mid-section emission: gated on this section's m==5 tile so
                # the scheduler cannot hoist the LN ahead of phase work whose
                # stall would head-block the vector FIFO
                ln_mid[ln_e].append(b)
            else:
                ln_end[ln_e].append(b)
            # out-write another expert later: by then nrm is guaranteed
            # ready, so the issue never stalls the scalar queue's head
            out_after[min(ln_e + 1, NUM_EXPERTS - 1)].append(b)
        t1pool = ctx.enter_context(tc.tile_pool(name="t1", bufs=TOK_BLOCKS))
        apool = ctx.enter_context(tc.tile_pool(name="ap", bufs=TOK_BLOCKS))
        nrmpool = ctx.enter_context(tc.tile_pool(name="nrm", bufs=TOK_BLOCKS))
        gatepool = ctx.enter_context(tc.tile_pool(name="gate", bufs=4))
        t1_tiles: dict = {}
        a_tiles: dict = {}
        nrm_tiles: dict = {}

        def gather_a(b: int):
            rows = int(growsa[b])
            t1 = t1pool.tile([128, EMBED], BF16, tag="t1")
            t1_tiles[b] = t1
            nc.gpsimd.indirect_dma_start(
                out=t1,
                out_offset=None,
                in_=ydram[:rows],
                in_offset=bass.IndirectOffsetOnAxis(ap=idx_t[:, b, 0:1], axis=0),
            )

        def gather_b(b: int):
            rows = int(growsb[b])
            t1 = t1_tiles[b]
            nc.gpsimd.indirect_dma_start(
                out=t1,
                out_offset=None,
                in_=ydram[:rows],
                in_offset=bass.IndirectOffsetOnAxis(ap=idx_t[:, b, 1:2], axis=0),
                compute_op=mybir.AluOpType.add,
            )

        def residual_add(b: int):
            # on gpsimd: waits on its own ring's gather (natural order)
            # instead of head-blocking the vector queue
            t1 = t1_tiles[b]
            a_t = apool.tile([128, EMBED], F32, tag="a")
            a_tiles[b] = a_t
            nc.gpsimd.tensor_add(out=a_t, in0=t1, in1=xres_t[:, b, :])

        def combine(b: int, gate=None):
            a_t = a_tiles[b]
            if gate is not None:
                # value-exact no-op (a[:,0:1] := 0*gate + a[:,0:1]): ties the
                # LN chain's readiness to this section's mid-phase-1 progress
                nc.vector.scalar_tensor_tensor(
                    out=a_t[:, 0:1], in0=gate, scalar=0.0, in1=a_t[:, 0:1],
                    op0=mybir.AluOpType.mult, op1=mybir.AluOpType.add,
                )
            stats = cpool.tile([128, 6], F32, tag="st")
            nc.vector.bn_stats(out=stats, in_=a_t)
            mv = cpool.tile([128, 2], F32, tag="mv")
            nc.vector.bn_aggr(out=mv, in_=stats)
            # rs = rsqrt(var + eps) entirely on DVE (keeps ScalarE's act
            # table pinned to Silu): bit-trick seed + 2 Newton steps
            ve = cpool.tile([128, 1], F32, tag="ve")
            nc.vector.tensor_scalar_add(out=ve, in0=mv[:, 1:2], scalar1=LN_EPS)
            hve = cpool.tile([128, 1], F32, tag="hve")
            nc.vector.tensor_scalar_mul(out=hve, in0=ve, scalar1=-0.5)
            rsi = cpool.tile([128, 1], I32, tag="rsi")
            nc.vector.tensor_scalar(
                out=rsi, in0=ve.bitcast(I32), scalar1=1, scalar2=None,
                op0=mybir.AluOpType.logical_shift_right,
            )
            nc.vector.tensor_scalar(
                out=rsi, in0=rsi, scalar1=-1, scalar2=None,
                op0=mybir.AluOpType.bitwise_xor,
            )
            nc.vector.tensor_scalar(
                out=rsi, in0=rsi, scalar1=0x5F3759DF + 1, scalar2=None,
                op0=mybir.AluOpType.add,
            )
            rs = rsi.bitcast(F32)
            t_ = cpool.tile([128, 1], F32, tag="tnw")
            for _ in range(2):
                nc.vector.tensor_mul(out=t_, in0=rs, in1=rs)
                nc.vector.tensor_scalar(
                    out=t_, in0=t_, scalar1=hve, scalar2=1.5,
                    op0=mybir.AluOpType.mult, op1=mybir.AluOpType.add,
                )
                nc.vector.tensor_mul(out=rs, in0=rs, in1=t_)
            nrm = nrmpool.tile([128, EMBED], F32, tag="nrm")
            nrm_tiles[b] = nrm
            nc.vector.tensor_scalar(
                out=nrm, in0=a_t, scalar1=mv[:, 0:1], scalar2=rs,
                op0=mybir.AluOpType.subtract, op1=mybir.AluOpType.mult,
            )
            if ln_affine:
                nc.vector.tensor_mul(out=nrm, in0=nrm, in1=gam_t)
                nc.vector.tensor_add(out=nrm, in0=nrm, in1=bet_t)

        for e in range(NUM_EXPERTS):
            C = caps[e]
            xt_t = xpool.tile([128, 4, C], FP8, tag="xt")
            b_t = wpool.tile([128, 2 * NTILE], F32, tag="bias")
            wv_t = wpool.tile([128, 4, HIDDEN_PAD], FP8, tag="wv")
            wg_t = wpool.tile([128, 4, HIDDEN_PAD], FP8, tag="wg")
            wo_t = wpool.tile([128, NTILE + 1, EMBED], FP8, tag="wo")
            o0 = int(off[e])
            if e == 0:
                # startup: spread the critical first loads across queues so
                # the first matmul pair can begin as early as possible
                nc.sync.dma_start(out=xt_t[:, :2], in_=xt[:, :2, o0 : o0 + C])
                nc.scalar.dma_start(out=wv_t[:, :1], in_=wv[e, :, :1])
                nc.gpsimd.dma_start(out=wv_t[:, 1:2], in_=wv[e, :, 1:2])
                nc.sync.dma_start(out=xt_t[:, 2:], in_=xt[:, 2:, o0 : o0 + C])
                nc.sync.dma_start(out=b_t, in_=bias[e])
                nc.scalar.dma_start(out=wv_t[:, 2:], in_=wv[e, :, 2:])
                nc.gpsimd.dma_start(out=wg_t[:, :2], in_=wg[e, :, :2])
                nc.scalar.dma_start(out=wg_t[:, 2:], in_=wg[e, :, 2:])
                nc.sync.dma_start(out=wo_t, in_=wo[e])
                nc.scalar.dma_start(out=idx_t, in_=idx[:, :, :])
                nc.scalar.dma_start(out=wsl_t, in_=wsl[:, :])
                nc.gpsimd.dma_start(out=xres_t, in_=xres[:, :, :])
                if ln_affine:
                    nc.scalar.dma_start(out=gam_t, in_=gam[:, :])
                    nc.scalar.dma_start(out=bet_t, in_=bet[:, :])
            elif e == 1:
                # scalar/gpsimd rings are still idle this early; keep the
                # sync ring short so e1's weights land before e0 compute ends
                nc.sync.dma_start(out=xt_t, in_=xt[:, :, o0 : o0 + C])
                nc.sync.dma_start(out=b_t, in_=bias[e])
                nc.scalar.dma_start(out=wv_t, in_=wv[e])
                nc.gpsimd.dma_start(out=wg_t, in_=wg[e])
                nc.sync.dma_start(out=wo_t, in_=wo[e])
            else:
                # steady state: the sync queue carries ONLY loads, so weight
                # prefetch never head-blocks behind compute-dependent DMAs
                nc.sync.dma_start(out=xt_t, in_=xt[:, :, o0 : o0 + C])
                nc.sync.dma_start(out=b_t, in_=bias[e])
                nc.sync.dma_start(out=wv_t, in_=wv[e])
                nc.sync.dma_start(out=wg_t, in_=wg[e])
                nc.sync.dma_start(out=wo_t, in_=wo[e])

            h_t = hpool.tile([128, NTILE + 1, C], FP8, tag="h")
            nc.vector.memset(h_t[:, NTILE, :], 1.0)

            for m in range(NTILE):
                psv = pvg.tile([128, cmax], F32, tag="psv")
                psg = pvg.tile([128, cmax], F32, tag="psg")
                for kk in range(2):
                    nc.tensor.matmul(
                        psv[:, :C],
                        lhsT=wv_t[:, 2 * kk : 2 * kk + 2, m * 128 : (m + 1) * 128],
                        rhs=xt_t[:, 2 * kk : 2 * kk + 2, :],
                        start=(kk == 0),
                        stop=(kk == 1),
                        perf_mode=DR,
                    )
                for kk in range(2):
                    nc.tensor.matmul(
                        psg[:, :C],
                        lhsT=wg_t[:, 2 * kk : 2 * kk + 2, m * 128 : (m + 1) * 128],
                        rhs=xt_t[:, 2 * kk : 2 * kk + 2, :],
                        start=(kk == 0),
                        stop=(kk == 1),
                        perf_mode=DR,
                    )
                v_t = vgpool.tile([128, C], BF16, tag="v")
                nc.scalar.activation(
                    out=v_t,
                    in_=psv[:, :C],
                    func=mybir.ActivationFunctionType.Silu,
                    bias=b_t[:, m : m + 1],
                    scale=1.0 / S_WV,
                )
                # h(fp8, x8) = (psg + 8*bg) * silu_v  -- one fused DVE pass
                nc.vector.scalar_tensor_tensor(
                    out=h_t[:, m, :],
                    in0=psg[:, :C],
                    scalar=b_t[:, NTILE + m : NTILE + m + 1],
                    in1=v_t,
                    op0=mybir.AluOpType.add,
                    op1=mybir.AluOpType.mult,
                )
                if m == 5 and ln_mid[e]:
                    # gate copy releases promptly (depends only on SILU m5),
                    # keeping the vgpool WAR chain off the gather path
                    gate_t = gatepool.tile([128, 1], BF16, tag="g")
                    nc.vector.tensor_scalar_mul(
                        out=gate_t, in0=v_t[:, 0:1], scalar1=1.0
                    )
                    with tc.high_priority(offset=-1000000):
                        for b in ln_mid[e]:
                            combine(b, gate=gate_t)
            for blk in range(blks[e]):
                mb = min(128, C - blk * 128)
                gblk = int(off2[e]) // 128 + blk
                pso = pop.tile([128, EMBED], F32, tag="pso")
                for kk in range(6):
                    nc.tensor.matmul(
                        pso[:mb],
                        lhsT=h_t[:, 2 * kk : 2 * kk + 2, blk * 128 : blk * 128 + mb],
                        rhs=wo_t[:, 2 * kk : 2 * kk + 2, :],
                        start=(kk == 0),
                        stop=(kk == 5),
                        perf_mode=DR,
                    )
                y_t = ypool.tile([128, EMBED], BF16, tag="y")
                if e == NUM_EXPERTS - 1:
                    # tail-critical path: ScalarE is idle by now and the
                    # ydram issue follows on the same engine, skipping the
                    # vector queue's backlog entirely
                    with tc.high_priority(offset=None):
                        nc.scalar.activation(
                            out=y_t[:mb],
                            in_=pso[:mb],
                            func=mybir.ActivationFunctionType.Identity,
                            bias=0.0,
                            scale=wsl_t[:mb, gblk : gblk + 1],
                        )
                else:
                    nc.vector.tensor_scalar_mul(
                        out=y_t[:mb], in0=pso[:mb], scalar1=wsl_t[:mb, gblk : gblk + 1]
                    )
                r0 = gblk * 128
                with tc.high_priority(offset=None) if e == NUM_EXPERTS - 1 else _null_ctx():
                    nc.scalar.dma_start(out=ydram[r0 : r0 + mb, :], in_=y_t[:mb])
                if mb < 128:
                    nc.scalar.dma_start(
                        out=ydram[r0 + mb : r0 + 128, :], in_=zero_t[: 128 - mb]
                    )
                with tc.high_priority(offset=-1000000):
                    for b in gather_a_at[gblk + 1]:
                        gather_a(b)
                    for b in gather_b_at[gblk + 1]:
                        gather_b(b)
                        residual_add(b)

            with tc.high_priority(offset=-1000000):
                for b in ln_end[e]:
                    combine(b)
                for b in out_after[e]:
                    nc.scalar.dma_start(out=out[b], in_=nrm_tiles[b])

    nc.finalize()
    _NC_CACHE[key] = nc
    return nc


def prepare(x, router_w, Wv, bv, Wg, bg, Wo, bo, gamma, beta):
    """Host-side routing, balancing, and per-core input construction."""
    x = np.asarray(x)
    router_w = np.asarray(router_w, dtype=np.float32)
    Wv = np.asarray(Wv, dtype=np.float32)
    bv = np.asarray(bv, dtype=np.float32)
    Wg = np.asarray(Wg, dtype=np.float32)
    bg = np.asarray(bg, dtype=np.float32)
    Wo = np.asarray(Wo, dtype=np.float32)
    bo = np.asarray(bo, dtype=np.float32)
    gamma = np.asarray(gamma, dtype=np.float32)
    beta = np.asarray(beta, dtype=np.float32)

    orig_shape = x.shape
    flat = x.reshape(-1, EMBED).astype(np.float32)
    n = flat.shape[0]
    assert n == NCORE * TOK_PER_CORE

    e1, e2, w1, w2 = _route(flat, router_w)
    # process experts in descending popularity: the LAST-processed expert
    # gates the tail combines, so make it the least-loaded one (fewest
    # dependent tokens -> fewer tail blocks)
    cnt = np.bincount(np.concatenate([e1, e2]), minlength=NUM_EXPERTS)
    perm = np.argsort(-cnt, kind="stable")  # position -> physical expert
    inv = np.empty(NUM_EXPERTS, np.int64)
    inv[perm] = np.arange(NUM_EXPERTS)
    e1 = inv[e1]
    e2 = inv[e2]
    Wv, bv = Wv[perm], bv[perm]
    Wg, bg = Wg[perm], bg[perm]
    Wo, bo = Wo[perm], bo[perm]
    assign, core_expert = _assign(e1, e2)

    # 16-aligned: DoubleRow LDWEIGHTS requires the k-pair stride (=C) % 16 == 0
    caps = tuple(int(-16 * (-core_expert[:, e].max() // 16)) for e in range(NUM_EXPERTS))
    blks = tuple((c + 127) // 128 for c in caps)
    off = np.concatenate([[0], np.cumsum(caps)]).astype(int)
    off2 = np.concatenate([[0], np.cumsum([b * 128 for b in blks])]).astype(int)
    ctot = int(off[-1])
    nblk = int(sum(blks))

    # replicated weights, pre-tiled to [e, 128, ktiles, free], fp8 pre-scaled
    def ktile(w, s):  # [E, 512, F] -> [E, 128, 4, F] fp8 of s*w
        E_, K_, F_ = w.shape
        return np.ascontiguousarray(
            _fp8(s * w).reshape(E_, 4, 128, F_).transpose(0, 2, 1, 3)
        )

    wv_r = np.zeros((NUM_EXPERTS, EMBED, HIDDEN_PAD), np.float32)
    wv_r[:, :, :HIDDEN_RAW] = Wv
    wg_r = np.zeros((NUM_EXPERTS, EMBED, HIDDEN_PAD), np.float32)
    wg_r[:, :, :HIDDEN_RAW] = Wg
    wv_tiled = ktile(wv_r, S_WV)
    wg_tiled = ktile(wg_r, S_WG)
    # Wo: [E, H, 512] -> tiles [E, 128, 12, 512]; tile 11 = bo (bias k-tile)
    wo_r = np.zeros((NUM_EXPERTS, (NTILE + 1) * 128, EMBED), np.float32)
    wo_r[:, :HIDDEN_RAW, :] = S_WO * Wo
    wo_r[:, NTILE * 128 :, :] = bo[:, None, :]  # all 128 rows = bo
    wo_tiled = np.ascontiguousarray(
        _fp8(wo_r).reshape(NUM_EXPERTS, NTILE + 1, 128, EMBED).transpose(0, 2, 1, 3)
    )
    bv_pad = np.zeros((NUM_EXPERTS, HIDDEN_PAD), np.float32)
    bv_pad[:, :HIDDEN_RAW] = bv
    bg_pad = np.zeros((NUM_EXPERTS, HIDDEN_PAD), np.float32)
    bg_pad[:, :HIDDEN_RAW] = bg
    # bias [e, 128, 22]: cols 0..10 = bv tiles, 11..21 = 8*bg tiles
    bias_h = np.concatenate(
        [
            bv_pad.reshape(NUM_EXPERTS, NTILE, 128).transpose(0, 2, 1),
            S_WG * bg_pad.reshape(NUM_EXPERTS, NTILE, 128).transpose(0, 2, 1),
        ],
        axis=2,
    ).astype(np.float32)
    bias_h = np.ascontiguousarray(bias_h)
    gam_rep = np.ascontiguousarray(np.broadcast_to(gamma, (128, EMBED)))
    bet_rep = np.ascontiguousarray(np.broadcast_to(beta, (128, EMBED)))

    # per-core token ordering by max expert touched -> per-block gather bounds
    emin_all = np.minimum(e1, e2)
    emax_all = np.maximum(e1, e2)
    per_core_sorted = []
    gbv = np.zeros(TOK_BLOCKS, np.int64)
    gbav = np.zeros(TOK_BLOCKS, np.int64)
    for c in range(NCORE):
        tok_c = np.nonzero(assign == c)[0]
        assert tok_c.size == TOK_PER_CORE
        emax = emax_all[tok_c]
        order = np.argsort(emax, kind="stable")
        tok_c = tok_c[order]
        per_core_sorted.append(tok_c)
        emax_sorted = emax[order]
        emin_sorted = emin_all[tok_c]
        for b in range(TOK_BLOCKS):
            gbv[b] = max(gbv[b], emax_sorted[(b + 1) * 128 - 1] + 1)
            gbav[b] = max(gbav[b], emin_sorted[b * 128 : (b + 1) * 128].max() + 1)
    gb = tuple(int(v) for v in gbv)
    gba = tuple(int(v) for v in gbav)

    in_maps = []
    core_token_ids = []
    growsa_v = np.zeros(TOK_BLOCKS, np.int64)
    growsb_v = np.zeros(TOK_BLOCKS, np.int64)
    for c in range(NCORE):
        tok_c = per_core_sorted[c]
        core_token_ids.append(tok_c)
        xt_c = np.zeros((EMBED, ctot), np.float32)
        ridx = np.zeros((TOK_PER_CORE, 2), np.int64)
        wslot = np.zeros(nblk * 128, np.float32)
        ids_by_e = []
        for e in range(NUM_EXPERTS):
            sel1 = e1[tok_c] == e
            sel2 = e2[tok_c] == e
            # slots ordered by token position (= owning combine block), so
            # per-block gather bounds only cover a sub-expert row prefix
            ids = np.nonzero(sel1 | sel2)[0]
            ids_by_e.append(ids)
            cnt = ids.size
            assert cnt <= caps[e], (cnt, caps[e])
            xt_c[:, off[e] : off[e] + cnt] = flat[tok_c[ids]].T
            rows = off2[e] + np.arange(cnt)
            # slot 0 = the token's min-expert row (gatherable early),
            # slot 1 = its max-expert row
            col = (emax_all[tok_c[ids]] == e).astype(np.int64)
            ridx[ids, col] = rows
            wslot[rows] = np.where(sel1[ids], w1[tok_c[ids]], w2[tok_c[ids]])
        emax_c = emax_all[tok_c]
        emin_c = emin_all[tok_c]
        for b in range(TOK_BLOCKS):
            hi = (b + 1) * 128
            eb = int(emax_c[b * 128 : hi].max())
            ea = int(emin_c[b * 128 : hi].max())
            growsb_v[b] = max(
                growsb_v[b], off2[eb] + np.searchsorted(ids_by_e[eb], hi)
            )
            growsa_v[b] = max(
                growsa_v[b], off2[ea] + np.searchsorted(ids_by_e[ea], hi)
            )
        in_map = {
            "xt": np.ascontiguousarray(
                _fp8(xt_c).reshape(4, 128, ctot).transpose(1, 0, 2)
            ),
            "wv": wv_tiled,
            "wg": wg_tiled,
            "wo": wo_tiled,
            "bias": bias_h,
            "gamma": gam_rep,
            "beta": bet_rep,
            "xres": np.ascontiguousarray(
                flat[tok_c].reshape(TOK_BLOCKS, 128, EMBED).transpose(1, 0, 2)
            ),
            "idx": np.ascontiguousarray(
                ridx.reshape(TOK_BLOCKS, 128, 2).transpose(1, 0, 2).astype(np.int32)
            ),
            "wsl": np.ascontiguousarray(
                (wslot / 128.0).reshape(nblk, 128).T.astype(np.float32)
            ),
        }
        in_maps.append(in_map)

    growsa = tuple(int(v) for v in growsa_v)
    growsb = tuple(int(v) for v in growsb_v)
    ln_affine = not (np.all(gamma == 1.0) and np.all(beta == 0.0))
    return in_maps, caps, gb, gba, growsa, growsb, ln_affine, core_token_ids, orig_shape


def assemble(results, core_token_ids, orig_shape):
    n = NCORE * TOK_PER_CORE
    out_full = np.zeros((n, EMBED), np.float32)
    for c in range(NCORE):
        out_full[core_token_ids[c]] = results[c]["out"].reshape(TOK_PER_CORE, EMBED)
    return out_full.reshape(orig_shape)


def kernel(x, router_w, Wv, bv, Wg, bg, Wo, bo, gamma, beta):
    in_maps, caps, gb, gba, growsa, growsb, ln_affine, core_token_ids, orig_shape = prepare(
        x, router_w, Wv, bv, Wg, bg, Wo, bo, gamma, beta
    )
    nc = _build_nc(caps, gb, gba, growsa, growsb, ln_affine)
    res = run_bass_kernel_spmd(nc, in_maps, list(range(NCORE)))
    return assemble(res.results, core_token_ids, orig_shape)
